# revision 1
# baseline (speedup 1.0000x reference)
"""Trainium2 Bass kernel for nn_JointPairHead: edge gather + LN + 3x(Linear->BN->ReLU) -> logits.

Sharding: data-parallel over E across 8 cores; x and params replicated.
BN batch stats cross-core via AllReduce of per-shard sum/sumsq.

Device dataflow (per core, E_shard = 32768 edges, 64 blocks of 512):
  P0: indirect-gather x[src]+x[dst] (natural layout [128e, 256d]) -> LayerNorm
      -> PE transpose to [256d, 512e] -> matmul z1 = W1' @ h0T (ln_w folded into W1)
      -> accumulate per-channel sum/sumsq -> spill z1T to DRAM
  AllReduce stats -> BN affine a,b
  P1/P2: read ziT, ACT relu-affine, matmul z_{i+1}, stats, spill
  P3: read z3T, relu-affine, matmul with w_out -> logits
"""

import numpy as np

N_NODES = 50000
D = 256
E_TOT = 262144
NCORES = 8
ESH = E_TOT // NCORES          # 32768 edges per core
EBLK = 512                      # edges per block
NBLK = ESH // EBLK              # 64
EPS = 1e-5
NL = 3
FP32 = None  # set on import of mybir inside build


def build_nc(n_blk=NBLK, num_devices=NCORES):
    import concourse.bass as bass
    import concourse.mybir as mybir
    import concourse.tile as tile
    from concourse import bacc
    from concourse.masks import make_identity

    f32 = mybir.dt.float32
    i32 = mybir.dt.int32
    A = mybir.ActivationFunctionType
    ALU = mybir.AluOpType
    AX = mybir.AxisListType

    esh = n_blk * EBLK
    groups = [list(range(num_devices))]

    nc = bacc.Bacc("TRN2", target_bir_lowering=False, debug=False,
                   num_devices=num_devices)

    # ---- kernel I/O ----
    x = nc.dram_tensor("x", [N_NODES, D], f32, kind="ExternalInput").ap()
    srci = nc.dram_tensor("srci", [128, n_blk * 4], i32, kind="ExternalInput").ap()
    dsti = nc.dram_tensor("dsti", [128, n_blk * 4], i32, kind="ExternalInput").ap()
    wts = [nc.dram_tensor(f"w{i}t", [D, D], f32, kind="ExternalInput").ap()
           for i in range(NL)]
    wot = nc.dram_tensor("wot", [D, 1], f32, kind="ExternalInput").ap()
    gam = nc.dram_tensor("gam", [NL, D], f32, kind="ExternalInput").ap()
    bet = nc.dram_tensor("bet", [NL, D], f32, kind="ExternalInput").ap()
    out = nc.dram_tensor("out", [esh], f32, kind="ExternalOutput").ap()

    # ---- internal DRAM: z spills and collective bounce buffers ----
    zt = [nc.dram_tensor(f"z{i}t", [2, 128, esh], f32, kind="Internal").ap()
          for i in range(NL)]
    ccin = [nc.dram_tensor(f"ccin{i}", [128, 4], f32, kind="Internal").ap()
            for i in range(NL)]
    cc_space = "Shared" if num_devices > 4 else "Local"
    ccout = [nc.dram_tensor(f"ccout{i}", [128, 4], f32, kind="Internal",
                            addr_space=cc_space).ap()
             for i in range(NL)]

    inv_d = 1.0 / D
    inv_e = 1.0 / (esh * num_devices)

    with tile.TileContext(nc) as tc:
        with (
            tc.tile_pool(name="const", bufs=1) as cpool,
            tc.tile_pool(name="io", bufs=3) as iop,
            tc.tile_pool(name="work", bufs=2) as wp,
            tc.tile_pool(name="stats", bufs=1) as sp,
            tc.tile_pool(name="psum", bufs=2, space="PSUM") as pp,
        ):
            # ---- constants / params in SBUF ----
            ident = cpool.tile([128, 128], f32, name="ident")
            make_identity(nc, ident[:])
            srci_sb = cpool.tile([128, n_blk * 4], i32, name="srci_sb")
            dsti_sb = cpool.tile([128, n_blk * 4], i32, name="dsti_sb")
            nc.sync.dma_start(out=srci_sb[:], in_=srci[:])
            nc.sync.dma_start(out=dsti_sb[:], in_=dsti[:])
            # weights: per layer, 2 k-chunk tiles [128k, 256j]
            wsb = []
            for i in range(NL):
                chunks = []
                for c in range(2):
                    t = cpool.tile([128, D], f32, name=f"w{i}c{c}")
                    nc.sync.dma_start(out=t[:], in_=wts[i][c * 128:(c + 1) * 128, :])
                    chunks.append(t)
                wsb.append(chunks)
            wot_sb = []
            for c in range(2):
                t = cpool.tile([128, 1], f32, name=f"wo{c}")
                nc.sync.dma_start(out=t[:], in_=wot[c * 128:(c + 1) * 128, :])
                wot_sb.append(t)
            gam_sb, bet_sb = [], []
            for i in range(NL):
                g = cpool.tile([128, 2], f32, name=f"gam{i}")
                b = cpool.tile([128, 2], f32, name=f"bet{i}")
                for c in range(2):
                    nc.sync.dma_start(out=g[:, c:c + 1],
                                      in_=gam[i, c * 128:(c + 1) * 128])
                    nc.sync.dma_start(out=b[:, c:c + 1],
                                      in_=bet[i, c * 128:(c + 1) * 128])
                gam_sb.append(g)
                bet_sb.append(b)

            # ---- per-layer stat accumulators and BN affine params ----
            Sz = [[sp.tile([128, n_blk], f32, name=f"Sz{i}_{j}") for j in range(2)]
                  for i in range(NL)]
            SSz = [[sp.tile([128, n_blk], f32, name=f"SSz{i}_{j}") for j in range(2)]
                   for i in range(NL)]
            a_ab = [sp.tile([128, 2], f32, name=f"a{i}") for i in range(NL)]
            b_ab = [sp.tile([128, 2], f32, name=f"b{i}") for i in range(NL)]

            def produce_z(li, blk, rhs0, rhs1):
                """matmul z_li = W_li @ h, accumulate stats, spill to DRAM."""
                rhs = [rhs0, rhs1]
                for j in range(2):
                    zps = pp.tile([128, EBLK], f32, name="zps", tag="zps")
                    for c in range(2):
                        nc.tensor.matmul(
                            out=zps[:],
                            lhsT=wsb[li][c][:, j * 128:(j + 1) * 128],
                            rhs=rhs[c][:],
                            start=(c == 0), stop=(c == 1))
                    zsb = wp.tile([128, EBLK], f32, name="zsb", tag="zsb")
                    nc.scalar.activation(out=zsb[:], in_=zps[:], func=A.Copy,
                                         accum_out=Sz[li][j][:, blk:blk + 1])
                    zsq = wp.tile([128, EBLK], f32, name="zsq", tag="zsq")
                    nc.vector.scalar_tensor_tensor(
                        out=zsq[:], in0=zsb[:], scalar=1.0, in1=zsb[:],
                        op0=ALU.mult, op1=ALU.mult,
                        accum_out=SSz[li][j][:, blk:blk + 1])
                    nc.sync.dma_start(
                        out=zt[li][j, :, blk * EBLK:(blk + 1) * EBLK], in_=zsb[:])

            # ================= Phase 0: gather + LN + layer 0 =================
            for blk in range(n_blk):
                xs = iop.tile([128, 4 * D], f32, name="xs", tag="xs")
                xd = iop.tile([128, 4 * D], f32, name="xd", tag="xd")
                # multi-column offset APs are broken on HW (walrus unroll);
                # one offset column (128 rows) per indirect DMA.
                for g in range(4):
                    col = blk * 4 + g
                    nc.gpsimd.indirect_dma_start(
                        out=xs[:, g * D:(g + 1) * D], out_offset=None, in_=x[:, :],
                        in_offset=bass.IndirectOffsetOnAxis(
                            ap=srci_sb[:, col:col + 1], axis=0))
                    nc.gpsimd.indirect_dma_start(
                        out=xd[:, g * D:(g + 1) * D], out_offset=None, in_=x[:, :],
                        in_offset=bass.IndirectOffsetOnAxis(
                            ap=dsti_sb[:, col:col + 1], axis=0))
                h = wp.tile([128, 4 * D], f32, name="h", tag="h")
                nc.vector.tensor_add(out=h[:], in0=xs[:], in1=xd[:])
                # --- LayerNorm over feature dim (per 128-edge group) ---
                h3 = h[:].rearrange("p (g d) -> p g d", d=D)
                Sln = wp.tile([128, 4], f32, name="Sln", tag="Sln")
                SSln = wp.tile([128, 4], f32, name="SSln", tag="SSln")
                nc.vector.reduce_sum(out=Sln[:], in_=h3, axis=AX.X)
                lsc = wp.tile([128, D], f32, name="lsc", tag="lsc")
                for g in range(4):
                    nc.scalar.activation(out=lsc[:], in_=h3[:, g, :], func=A.Square,
                                         accum_out=SSln[:, g:g + 1])
                mu = wp.tile([128, 4], f32, name="mu", tag="mu")
                mu2 = wp.tile([128, 4], f32, name="mu2", tag="mu2")
                var = wp.tile([128, 4], f32, name="var", tag="var")
                inv = wp.tile([128, 4], f32, name="inv", tag="inv")
                rs = wp.tile([128, 4], f32, name="rs", tag="rs")
                bneg = wp.tile([128, 4], f32, name="bneg", tag="bneg")
                nc.scalar.mul(out=mu[:], in_=Sln[:], mul=inv_d)
                nc.scalar.square(out=mu2[:], in_=mu[:])
                nc.vector.scalar_tensor_tensor(
                    out=var[:], in0=SSln[:], scalar=inv_d, in1=mu2[:],
                    op0=ALU.mult, op1=ALU.subtract)
                nc.vector.tensor_scalar_add(out=var[:], in0=var[:], scalar1=EPS)
                nc.vector.reciprocal(out=inv[:], in_=var[:])
                nc.scalar.sqrt(out=rs[:], in_=inv[:])
                nc.vector.scalar_tensor_tensor(
                    out=bneg[:], in0=mu[:], scalar=-1.0, in1=rs[:],
                    op0=ALU.mult, op1=ALU.mult)
                hn = wp.tile([128, 4 * D], f32, name="hn", tag="hn")
                for g in range(4):
                    nc.scalar.activation(
                        out=hn[:, g * D:(g + 1) * D], in_=h[:, g * D:(g + 1) * D],
                        func=A.Identity, bias=bneg[:, g:g + 1], scale=rs[:, g:g + 1])
                # --- transpose to [256d, 512e] ---
                hT = []
                for c in range(2):
                    tp = pp.tile([128, EBLK], f32, name="tp", tag="tp")
                    for g in range(4):
                        nc.tensor.transpose(
                            out=tp[:, g * 128:(g + 1) * 128],
                            in_=hn[:, g * D + c * 128: g * D + (c + 1) * 128],
                            identity=ident[:])
                    hc = wp.tile([128, EBLK], f32, name=f"hTc{c}", tag=f"hTc{c}")
                    nc.scalar.copy(out=hc[:], in_=tp[:])
                    hT.append(hc)
                produce_z(0, blk, hT[0], hT[1])

            # ============ stats AllReduce + BN affine, then layers 1..3 ============
            def finalize_stats(li):
                st4 = sp.tile([128, 4], f32, name=f"st4_{li}")
                for j in range(2):
                    nc.vector.reduce_sum(out=st4[:, j:j + 1], in_=Sz[li][j][:],
                                         axis=AX.X)
                    nc.vector.reduce_sum(out=st4[:, 2 + j:3 + j], in_=SSz[li][j][:],
                                         axis=AX.X)
                nc.sync.dma_start(out=ccin[li][:, :], in_=st4[:])
                if num_devices == 1:
                    # collective-free variant for TimelineSim profiling
                    nc.sync.dma_start(out=ccout[li][:, :], in_=ccin[li][:, :])
                else:
                    nc.gpsimd.collective_compute(
                        "AllReduce", ALU.add, replica_groups=groups,
                        ins=[ccin[li][:, :]], outs=[ccout[li][:, :]])
                gst = sp.tile([128, 4], f32, name=f"gst{li}")
                nc.sync.dma_start(out=gst[:], in_=ccout[li][:, :])
                bmu = sp.tile([128, 2], f32, name=f"bmu{li}")
                bmu2 = sp.tile([128, 2], f32, name=f"bmu2{li}")
                bvar = sp.tile([128, 2], f32, name=f"bvar{li}")
                binv = sp.tile([128, 2], f32, name=f"binv{li}")
                brs = sp.tile([128, 2], f32, name=f"brs{li}")
                tt = sp.tile([128, 2], f32, name=f"tt{li}")
                nc.scalar.mul(out=bmu[:], in_=gst[:, 0:2], mul=inv_e)
                nc.scalar.square(out=bmu2[:], in_=bmu[:])
                nc.vector.scalar_tensor_tensor(
                    out=bvar[:], in0=gst[:, 2:4], scalar=inv_e, in1=bmu2[:],
                    op0=ALU.mult, op1=ALU.subtract)
                nc.vector.tensor_scalar_add(out=bvar[:], in0=bvar[:], scalar1=EPS)
                nc.vector.reciprocal(out=binv[:], in_=bvar[:])
                nc.scalar.sqrt(out=brs[:], in_=binv[:])
                nc.vector.tensor_mul(out=a_ab[li][:], in0=gam_sb[li][:], in1=brs[:])
                nc.vector.tensor_mul(out=tt[:], in0=a_ab[li][:], in1=bmu[:])
                nc.vector.tensor_sub(out=b_ab[li][:], in0=bet_sb[li][:], in1=tt[:])

            finalize_stats(0)

            for li in range(1, NL):
                for blk in range(n_blk):
                    hT = []
                    for c in range(2):
                        zrd = iop.tile([128, EBLK], f32, name="zrd", tag="zrd")
                        nc.sync.dma_start(
                            out=zrd[:],
                            in_=zt[li - 1][c, :, blk * EBLK:(blk + 1) * EBLK])
                        hc = wp.tile([128, EBLK], f32, name=f"rhc{c}", tag=f"rhc{c}")
                        nc.scalar.activation(
                            out=hc[:], in_=zrd[:], func=A.Relu,
                            bias=b_ab[li - 1][:, c:c + 1],
                            scale=a_ab[li - 1][:, c:c + 1])
                        hT.append(hc)
                    produce_z(li, blk, hT[0], hT[1])
                finalize_stats(li)

            # ================= Phase 3: final projection =================
            for blk in range(n_blk):
                lps = pp.tile([1, EBLK], f32, name="lps", tag="lps")
                for c in range(2):
                    zrd = iop.tile([128, EBLK], f32, name="zrd3", tag="zrd3")
                    nc.sync.dma_start(
                        out=zrd[:], in_=zt[NL - 1][c, :, blk * EBLK:(blk + 1) * EBLK])
                    hc = wp.tile([128, EBLK], f32, name=f"fhc{c}", tag=f"fhc{c}")
                    nc.scalar.activation(
                        out=hc[:], in_=zrd[:], func=A.Relu,
                        bias=b_ab[NL - 1][:, c:c + 1], scale=a_ab[NL - 1][:, c:c + 1])
                    nc.tensor.matmul(out=lps[:], lhsT=wot_sb[c][:], rhs=hc[:],
                                     start=(c == 0), stop=(c == 1))
                lsb = wp.tile([1, EBLK], f32, name="lsb", tag="lsb")
                nc.scalar.copy(out=lsb[:], in_=lps[:])
                nc.sync.dma_start(out=out[blk * EBLK:(blk + 1) * EBLK], in_=lsb[:])

    nc.compile()
    return nc


_NC = None


def _prep_idx(idx, n_blk):
    # edge e = blk*512 + g*128 + p  ->  column blk*4+g, partition p
    return np.ascontiguousarray(
        idx.reshape(n_blk, 4, 128).transpose(2, 0, 1).reshape(128, n_blk * 4)
    ).astype(np.int32)


def kernel(**inputs):
    global _NC
    from concourse import bass_utils

    x = np.ascontiguousarray(np.asarray(inputs["x"], dtype=np.float32))
    ei = np.asarray(inputs["jg_edge_index"])
    ln_w = np.asarray(inputs["ln_w"], dtype=np.float32)
    Ws = np.asarray(inputs["Ws"], dtype=np.float32)
    gammas = np.asarray(inputs["gammas"], dtype=np.float32)
    betas = np.asarray(inputs["betas"], dtype=np.float32)
    W_out = np.asarray(inputs["W_out"], dtype=np.float32)

    # fold ln_w into layer-0 weight; lhsT layout = W.T ([in,out])
    W0f = Ws[0] * ln_w[None, :]
    wts = [np.ascontiguousarray(W0f.T), np.ascontiguousarray(Ws[1].T),
           np.ascontiguousarray(Ws[2].T)]
    wot = np.ascontiguousarray(W_out.reshape(1, D).T)  # [256,1]

    if _NC is None:
        _NC = build_nc()

    in_maps = []
    for c in range(NCORES):
        sl = slice(c * ESH, (c + 1) * ESH)
        in_maps.append({
            "x": x,
            "srci": _prep_idx(ei[0, sl], NBLK),
            "dsti": _prep_idx(ei[1, sl], NBLK),
            "w0t": wts[0].astype(np.float32),
            "w1t": wts[1].astype(np.float32),
            "w2t": wts[2].astype(np.float32),
            "wot": wot.astype(np.float32),
            "gam": gammas,
            "bet": betas,
        })
    global _last_in_maps
    _last_in_maps = in_maps
    res = bass_utils.run_bass_kernel_spmd(_NC, in_maps, core_ids=list(range(NCORES)))
    return np.concatenate([res.results[c]["out"] for c in range(NCORES)], axis=0)


_last_in_maps = None

